# revision 2
# baseline (speedup 1.0000x reference)
"""Trainium2 Bass kernel for nn_CrossGraphDA (retrieval_knn).

The reference computes, per branch b in {x1, x2}:
    h = Lin(x_b); Q,K = Lin(h); top-6 attention kNN graph; 2x SAGEConv+BN+ReLU
then G = Conv1x1(concat(f1, f2)), and finally
    x3n = 2*x3 - G ; x4n = 2*x4 - G
    delta = mean(x3n, 0) - mean(x4n, 0) ; out = dot(delta, delta)

Because BOTH x3n and x4n subtract the SAME G, G cancels exactly in delta:
    delta = 2*(mean(x3, 0) - mean(x4, 0))
This is a structural algebraic identity (holds for any inputs/weights), so
the whole GNN is dead code w.r.t. the scalar output; only column sums of
x3 and x4 survive.  Verified against the float32 reference: rel err ~1e-7.

Distribution: an 8-core AllReduce of per-shard partial sums measured ~65us
of collective/skew latency — far more than the whole computation — so every
core redundantly computes the full result from the full x3/x4 and the host
takes core 0's scalar (no cross-core dependency).

Per-core implementation (raw bass, no TileContext — measured ~1.5us faster
than the tile version: leaner entry/exit and full control of the sync):
  - two 1MB HWDGE loads on the sync queue ([128, 2048] images; partition p
    holds rows 64p..64p+63 as one contiguous 8KB descriptor).  Measured
    at ~360 GB/s = the per-NC HBM limit; chunking the loads was measured
    SLOWER (per-DMA completion-sem retirement grows ~1us per extra queued
    DMA, and finer pieces add DVE tree work).
  - DVE halving-add tree per tensor 2048 -> 64 cols (x3's tree and its two
    matmuls hide under x4's DMA; x4's last level negates via
    scalar_tensor_tensor so every matmul uses the same +1 weights and the
    framework's const-AP [128,1] 1.0f vector is reused — no memset).
  - four matmuls accumulate +t3/-t4 halves into one [1, 32] PSUM group.
  - PSUM -> SBUF copy, fused square+scale+sum (scalar_tensor_tensor with
    accum_out) -> out[1,1], stored from the sync queue.
  - NO explicit wait on the output DMA: the cleanup's gpsimd dma_reset is a
    *drain* (waits for in-flight DMAs), and the NEFF's fixed ~7us event-sync
    tail runs after the last instruction anyway, so the 4-byte write always
    lands before the runtime returns the buffer (~0.7us saved vs waiting).

Measured: ~20.9us HW exec (was 23.9us baseline), rel err ~3e-7.
"""

import numpy as np

import concourse.bass as bass
import concourse.mybir as mybir
from concourse import bacc
from concourse.bass_utils import run_bass_kernel_spmd

N_CORES = 8
N = 8192
D = 32
P = 128                      # SBUF partitions
W_FULL = (N // P) * D        # 2048 cols per partition per tensor
_F32 = mybir.dt.float32

# toggled by test.py only; the grading path never sets it
TRACE = False

_cached_nc = None


def _build():
    nc = bacc.Bacc(
        "TRN2",
        target_bir_lowering=False,
        debug=False,
        num_devices=N_CORES,
    )
    x3 = nc.dram_tensor("x3", [N, D], _F32, kind="ExternalInput")
    x4 = nc.dram_tensor("x4", [N, D], _F32, kind="ExternalInput")
    out = nc.dram_tensor("out", [1, 1], _F32, kind="ExternalOutput")

    src3 = x3.ap().rearrange("(p n) d -> p (n d)", p=P)
    src4 = x4.ap().rearrange("(p n) d -> p (n d)", p=P)

    with nc.cleanup_on_exit():
        ch3 = nc.alloc_sbuf_tensor("ch3", [P, W_FULL], _F32)
        ch4 = nc.alloc_sbuf_tensor("ch4", [P, W_FULL], _F32)
        ds = nc.alloc_sbuf_tensor("ds", [1, D], _F32)
        sq = nc.alloc_sbuf_tensor("sq", [1, D], _F32)
        res = nc.alloc_sbuf_tensor("res", [1, 1], _F32)
        ps = nc.alloc_psum_tensor("ps", [1, D], _F32)
        sem3 = nc.alloc_semaphore("sem3")
        sem4 = nc.alloc_semaphore("sem4")
        sem_t = nc.alloc_semaphore("sem_t")
        sem_mm = nc.alloc_semaphore("sem_mm")
        sem_v = nc.alloc_semaphore("sem_v")
        sem_out = nc.alloc_semaphore("sem_out")

        nc.sync.dma_start(ch3[:, :], src3).then_inc(sem3, 16)
        nc.sync.dma_start(ch4[:, :], src4).then_inc(sem4, 16)

        ones = nc.const_aps.aps[(_F32, 1.0)]  # [128, 1] of 1.0f, preamble-set

        def tree(src_t, dma_sem, neg, tag):
            nc.vector.wait_ge(dma_sem, 16)
            cur, width = src_t[:, :], W_FULL
            last = None
            while width > 2 * D:
                width //= 2
                nxt = nc.alloc_sbuf_tensor(f"tr_{tag}_{width}", [P, width], _F32)
                if neg and width == 2 * D:
                    # out = (in0 * -1) - in1 = -(left + right)
                    last = nc.vector.scalar_tensor_tensor(
                        out=nxt[:, :],
                        in0=cur[:, :width],
                        scalar=-1.0,
                        in1=cur[:, width:],
                        op0=mybir.AluOpType.mult,
                        op1=mybir.AluOpType.subtract,
                    )
                else:
                    last = nc.vector.tensor_add(
                        out=nxt[:, :], in0=cur[:, :width], in1=cur[:, width:]
                    )
                cur = nxt[:, :]
            last.then_inc(sem_t, 1)
            return cur  # [P, 2*D]

        t3 = tree(ch3, sem3, neg=False, tag="3")
        t4 = tree(ch4, sem4, neg=True, tag="4")

        nc.tensor.wait_ge(sem_t, 1)
        nc.tensor.matmul(out=ps[:, :], lhsT=ones, rhs=t3[:, :D], start=True, stop=False)
        nc.tensor.matmul(out=ps[:, :], lhsT=ones, rhs=t3[:, D:], start=False, stop=False)
        nc.tensor.wait_ge(sem_t, 2)
        nc.tensor.matmul(out=ps[:, :], lhsT=ones, rhs=t4[:, :D], start=False, stop=False)
        nc.tensor.matmul(
            out=ps[:, :], lhsT=ones, rhs=t4[:, D:], start=False, stop=True
        ).then_inc(sem_mm, 1)

        # out = dot(delta, delta) = sum((ps * (2/N)^2) * ps)
        nc.vector.wait_ge(sem_mm, 1)
        nc.vector.tensor_copy(ds[:, :], ps[:, :])
        nc.vector.scalar_tensor_tensor(
            out=sq[:, :],
            in0=ds[:, :],
            scalar=(2.0 / N) ** 2,
            in1=ds[:, :],
            op0=mybir.AluOpType.mult,
            op1=mybir.AluOpType.mult,
            accum_out=res[:, :],
        ).then_inc(sem_v, 1)

        nc.sync.wait_ge(sem_v, 1)
        nc.sync.dma_start(out.ap(), res[:, :]).then_inc(sem_out, 16)
        nc.all_engine_barrier()

    nc.compile()
    return nc


def kernel(**inputs) -> np.ndarray:
    global _cached_nc
    x3 = np.ascontiguousarray(np.asarray(inputs["x3"], dtype=np.float32))
    x4 = np.ascontiguousarray(np.asarray(inputs["x4"], dtype=np.float32))
    assert x3.shape == (N, D) and x4.shape == (N, D)

    if _cached_nc is None:
        _cached_nc = _build()

    in_maps = [{"x3": x3, "x4": x4} for _ in range(N_CORES)]
    r = run_bass_kernel_spmd(
        _cached_nc, in_maps, core_ids=list(range(N_CORES)), trace=TRACE
    )
    if TRACE:
        kernel.last_results = r
    val = np.asarray(r.results[0]["out"], dtype=np.float32).reshape(())
    return val
